# Initial kernel scaffold
#
"""Trainium2 Bass kernel for nn_AttentionBlock (GroupNorm + 1x1conv + MHA + residual).

Strategy:
  - Data-parallel over batch: 16 batches -> 8 NeuronCores x 2 batches. No collectives.
  - Host-side: fuse the 1x1 conv into the Q/K/V projections (float64 algebra),
    pre-transpose all weights to [in, out] and pre-round them to fp32r
    (11-bit-mantissa float the PE consumes at full speed, 1 col/cycle).
  - On-chip layout is channel-major [C, L] per batch. Attention computes
    S^T = K^T Q directly (j on partitions), exp on ScalarE straight out of
    PSUM over [128,1024] tiles (scores are bounded, so no max-subtraction is
    needed), and the softmax denominator comes for free as a ones-row
    appended to V^T in the AV matmul. Per-column normalization is done by
    bouncing the reciprocal row through DRAM with a partition-broadcast DMA.
    All matmuls are fp32r (rel err ~1e-4).
"""

import numpy as np

import concourse.bass as bass
import concourse.tile as tile
from concourse import bacc, mybir
from concourse.bass_utils import run_bass_kernel_spmd

P = 128
C = 512
L = 1024
B = 2          # batches per core
NCORES = 8
NH = 8
DK = 64
NCH = C // P   # 4 channel chunks of 128
GPC = 8        # groups per 128-chunk (16 channels per group)
GSIZE = 16
EPS = 1e-5
SCALE = float(DK) ** -0.5
F32 = mybir.dt.float32
F32R = mybir.dt.float32r
AO = mybir.AluOpType


def _round_fp32r(a: np.ndarray) -> np.ndarray:
    """Round-to-nearest-even to 11-bit mantissa (fp32r), keep fp32 layout."""
    b = np.ascontiguousarray(a, np.float32).view(np.uint32)
    r = (b.astype(np.uint64) + 0x7FF + ((b >> 12) & 1)).astype(np.uint32)
    return (r & np.uint32(0xFFFFF000)).view(np.float32)


def _build(flags, reps=1):
    has_bq, has_bk, has_bv, has_bo, has_gnw, has_gnb = flags
    nc = bacc.Bacc("TRN2", target_bir_lowering=False)

    x_d = nc.dram_tensor("x", [B, C, L], F32, kind="ExternalInput")
    wq_d = nc.dram_tensor("wq", [C, C], F32, kind="ExternalInput")   # [in, out], fp32r bits
    wk_d = nc.dram_tensor("wk", [C, C], F32, kind="ExternalInput")
    wv_d = nc.dram_tensor("wv", [C, C], F32, kind="ExternalInput")
    wo_d = nc.dram_tensor("wo", [C, C], F32, kind="ExternalInput")
    # params rows: 0 gn_w, 1 gn_b, 2 bq, 3 bk, 4 bv, 5 bo
    par_d = nc.dram_tensor("par", [6, C], F32, kind="ExternalInput")
    ones_d = nc.dram_tensor("ones", [P, GPC], F32, kind="ExternalInput")
    onesrow_d = nc.dram_tensor("onesrow", [1, DK], F32, kind="ExternalInput")
    gnsel_d = nc.dram_tensor("gnsel", [P, GPC], F32, kind="ExternalInput")
    gnbsel_d = nc.dram_tensor("gnbsel", [GPC, P], F32, kind="ExternalInput")
    out_d = nc.dram_tensor("out", [B, C, L], F32, kind="ExternalOutput")

    with tile.TileContext(nc) as tc:
        with (
            nc.allow_low_precision(reason="fp32r rounding of matmul inputs is intentional"),
            tc.tile_pool(name="weights", bufs=1) as wpool,
            tc.tile_pool(name="xpool", bufs=1) as xpool,
            tc.tile_pool(name="acts", bufs=1) as apool,
            tc.tile_pool(name="qk2", bufs=2) as qkpool,
            tc.tile_pool(name="xn", bufs=1) as xnpool,
            tc.tile_pool(name="pt", bufs=5) as ptpool,
            tc.tile_pool(name="small", bufs=1) as spool,
            tc.tile_pool(name="osb", bufs=5) as opool,
            tc.tile_pool(name="recb", bufs=4) as rpool,
            tc.tile_pool(name="ps_big", bufs=2, space="PSUM") as ps_big,
            tc.tile_pool(name="ps_av", bufs=2, space="PSUM") as ps_av,
            tc.tile_pool(name="ps_proj", bufs=2, space="PSUM") as ps_proj,
            tc.tile_pool(name="dram", bufs=4, space="DRAM") as dpool,
        ):
            # ---- x[b0] first (GN gates the pipeline), then q/k weights, then x[b1]
            x_t = []
            for b in range(B):
                xt = xpool.tile([P, NCH, L], F32, tag=f"x{b}")
                x_t.append(xt)

            def load_x(b):
                xr = x_d[b].rearrange("(k p) l -> p k l", p=P)
                for c in range(NCH):
                    nc.sync.dma_start(x_t[b][:, c, :], xr[:, c, :])

            load_x(0)
            w_t = {}
            for name, d in (("q", wq_d), ("k", wk_d)):
                w = wpool.tile([P, NCH, C], F32R, tag=f"w{name}")
                nc.sync.dma_start(w[:], d.rearrange("(k p) o -> p k o", p=P).bitcast(F32R))
                w_t[name] = w
            load_x(1)
            par = spool.tile([P, 6, NCH], F32, tag="par")
            nc.sync.dma_start(par[:], par_d.rearrange("j (k p) -> p j k", p=P))
            ones8 = spool.tile([P, GPC], F32, tag="ones8")
            nc.sync.dma_start(ones8[:], ones_d[:, :])
            onesrow = spool.tile([1, DK], F32R, tag="onesrow")
            nc.sync.dma_start(onesrow[:], onesrow_d[:, :].bitcast(F32R))
            gnsel = spool.tile([P, GPC], F32R, tag="gnsel")
            nc.sync.dma_start(gnsel[:], gnsel_d[:, :].bitcast(F32R))
            gnbsel = spool.tile([GPC, P], F32R, tag="gnbsel")
            nc.sync.dma_start(gnbsel[:], gnbsel_d[:, :].bitcast(F32R))
            eps8 = spool.tile([GPC, 1], F32, tag="eps8")
            nc.vector.memset(eps8[:], EPS)

            for rep in range(reps):
                # ---- GroupNorm statistics (per batch) ----
                rstd_pc, mean_pc, beta_pc = [], [], []
                for b in range(B):
                    # raw sums: DVE sum(x), ACT sum(x^2) via accum_out
                    rhs_f = spool.tile([P, 2 * NCH], F32, tag="gnrhs_f", name="rhs_f")
                    for c in range(NCH):
                        nc.vector.reduce_sum(rhs_f[:, c:c + 1], x_t[b][:, c, :],
                                             axis=mybir.AxisListType.X)
                        scr = ptpool.tile([P, L], F32, tag="pt", name="scr")
                        nc.scalar.activation(scr[:], x_t[b][:, c, :],
                                             mybir.ActivationFunctionType.Square,
                                             accum_out=rhs_f[:, NCH + c:NCH + c + 1])
                    rhs_r = spool.tile([P, 2 * NCH], F32R, tag="gnrhs_r", name="rhs_r")
                    nc.vector.tensor_copy(rhs_r[:], rhs_f[:])

                    gstat = ps_proj.tile([P, 512], F32, tag="proj",
                                         name="gstat")[0:GPC, 0:2 * NCH]
                    nc.tensor.matmul(gstat[:], gnsel[:], rhs_r[:], start=True, stop=True)

                    gmean = spool.tile([GPC, NCH], F32, tag="gmean", name="gmean")
                    nc.scalar.mul(gmean[:], gstat[:, 0:NCH], 1.0 / (GSIZE * L))
                    gm2 = spool.tile([GPC, NCH], F32, tag="gm2", name="gm2")
                    nc.vector.tensor_mul(gm2[:], gmean[:], gmean[:])
                    gvar = spool.tile([GPC, NCH], F32, tag="gvar", name="gvar")
                    nc.vector.scalar_tensor_tensor(
                        out=gvar[:], in0=gstat[:, NCH:2 * NCH], scalar=1.0 / (GSIZE * L),
                        in1=gm2[:], op0=AO.mult, op1=AO.subtract)
                    bvals = spool.tile([GPC, 2 * NCH], F32R, tag="bvals", name="bvals")
                    gstd = spool.tile([GPC, NCH], F32, tag="gstd", name="gstd")
                    nc.scalar.activation(gstd[:], gvar[:],
                                         mybir.ActivationFunctionType.Sqrt,
                                         bias=eps8[:], scale=1.0)
                    nc.vector.reciprocal(bvals[:, 0:NCH], gstd[:])
                    nc.vector.tensor_copy(bvals[:, NCH:2 * NCH], gmean[:])

                    bc = ps_proj.tile([P, 512], F32, tag="proj", name="bc")[:, 0:2 * NCH]
                    nc.tensor.matmul(bc[:], gnbsel[:], bvals[:], start=True, stop=True)
                    rp = spool.tile([P, NCH], F32, tag=f"rstd{b}", name="rp")
                    mp = spool.tile([P, NCH], F32, tag=f"mean{b}", name="mp")
                    if has_gnw:
                        nc.vector.tensor_tensor(rp[:], bc[:, 0:NCH], par[:, 0, :], AO.mult)
                    else:
                        nc.vector.tensor_copy(rp[:], bc[:, 0:NCH])
                    nc.scalar.copy(mp[:], bc[:, NCH:2 * NCH])
                    rstd_pc.append(rp)
                    mean_pc.append(mp)
                    if has_gnb:
                        bp = spool.tile([P, NCH], F32, tag=f"beta{b}", name="bp")
                        nc.vector.tensor_mul(bp[:], mp[:], rp[:])
                        nc.vector.tensor_tensor(bp[:], par[:, 1, :], bp[:], AO.subtract)
                        beta_pc.append(bp)
                    else:
                        beta_pc.append(None)

                # ================= pipeline =================
                prep = {}
                attns = {}

                def emit_proj(b):
                    # GN apply -> xn (fp32r)
                    xn = xnpool.tile([P, NCH, L], F32R, tag="xn", name="xn")
                    for c in range(NCH):
                        if has_gnb:
                            nc.vector.tensor_scalar(
                                out=xn[:, c, :], in0=x_t[b][:, c, :],
                                scalar1=rstd_pc[b][:, c:c + 1],
                                scalar2=beta_pc[b][:, c:c + 1],
                                op0=AO.mult, op1=AO.add)
                        else:
                            nc.vector.tensor_scalar(
                                out=xn[:, c, :], in0=x_t[b][:, c, :],
                                scalar1=mean_pc[b][:, c:c + 1],
                                scalar2=rstd_pc[b][:, c:c + 1],
                                op0=AO.subtract, op1=AO.mult)

                    # Q, K projections: [C, L] channel-major
                    qk = {}
                    for pname in ("q", "k"):
                        qk[pname] = qkpool.tile([P, NCH, L], F32R, tag=pname, name=pname)
                    for oc in range(NCH):  # interleave q/k so head 0 unblocks early
                        for pname, has_b, prow in (("q", has_bq, 2), ("k", has_bk, 3)):
                            dst = qk[pname]
                            for ih in range(2):
                                ps = ps_proj.tile([P, 512], F32, tag="proj", name="psp")
                                for ic in range(NCH):
                                    nc.tensor.matmul(
                                        ps[:], w_t[pname][:, ic, oc * P:(oc + 1) * P],
                                        xn[:, ic, ih * 512:(ih + 1) * 512],
                                        start=(ic == 0), stop=(ic == NCH - 1))
                                dv = dst[:, oc, ih * 512:(ih + 1) * 512]
                                if has_b:
                                    nc.vector.tensor_scalar(
                                        out=dv, in0=ps[:], scalar1=par[:, prow, oc:oc + 1],
                                        scalar2=0.0, op0=AO.add, op1=AO.bypass)
                                else:
                                    nc.vector.tensor_copy(dv, ps[:])

                    # V^T: [L, C] token-major with ones column per head
                    if b == 0:
                        wv_s = wpool.tile([P, NCH, C], F32R, tag="wvo", name="wv_s")
                        nc.sync.dma_start(
                            wv_s[:], wv_d.rearrange("(k p) o -> p k o", p=P).bitcast(F32R))
                        w_t["v"] = wv_s
                    vT = []
                    for lb in range(NH):
                        vt = apool.tile([P, NH, DK + 1], F32R, tag=f"vT{lb}", name="vt")
                        ps = ps_proj.tile([P, 512], F32, tag="proj", name="psv")
                        for ic in range(NCH):
                            nc.tensor.matmul(
                                ps[:], xn[:, ic, lb * P:(lb + 1) * P], w_t["v"][:, ic, :],
                                start=(ic == 0), stop=(ic == NCH - 1))
                        nc.vector.tensor_copy(vt[:, :, 0:DK],
                                               ps[:].rearrange("p (h d) -> p h d", d=DK))
                        nc.vector.tensor_copy(vt[:, :, DK], ones8[:])
                        vT.append(vt)
                    prep[b] = (qk, vT)

                def emit_attn(b):
                    qk, vT = prep[b]
                    attn = apool.tile([P, NCH, L], F32R, tag="attn", name="attn")
                    q_t, k_t = qk["q"], qk["k"]
                    for h in range(NH):
                        hb = (h % 2) * DK
                        t = h // 2
                        avs = [ps_av.tile([P, 512], F32, tag="av", name=f"av{ih}")
                               for ih in range(2)]
                        for jb in range(NH):
                            sps = ps_big.tile([P, L], F32, tag="s", name="sps")
                            for ih in range(2):
                                nc.tensor.matmul(
                                    sps[:, ih * 512:(ih + 1) * 512],
                                    k_t[hb:hb + DK, t, jb * P:(jb + 1) * P],
                                    q_t[hb:hb + DK, t, ih * 512:(ih + 1) * 512],
                                    start=True, stop=True)
                            pt = ptpool.tile([P, L], F32R, tag="pt", name="pt")
                            nc.scalar.activation(pt[:], sps[:],
                                                 mybir.ActivationFunctionType.Exp,
                                                 scale=SCALE)
                            for ih in range(2):
                                nc.tensor.matmul(
                                    avs[ih][0:DK + 1, :], vT[jb][:, h, :],
                                    pt[:, ih * 512:(ih + 1) * 512],
                                    start=(jb == 0), stop=(jb == NH - 1))
                        for ih in range(2):
                            av = avs[ih]
                            rec = rpool.tile([1, 512], F32, tag="rb", name="rec")
                            nc.vector.reciprocal(rec[:], av[DK:DK + 1, :])
                            av_view = attn[hb:hb + DK, t, ih * 512:(ih + 1) * 512]
                            nc.vector.tensor_copy(av_view, av[0:DK, :])
                            scr = dpool.tile([1, 512], F32, tag="scr", name="scr")
                            nc.sync.dma_start(scr[:], rec[:])
                            recb = rpool.tile([P, 512], F32, tag="rb", name="recb")
                            nc.sync.dma_start(recb[:], scr[:].to_broadcast((P, 512)))
                            nc.vector.tensor_tensor(av_view, av_view,
                                                    recb[hb:hb + DK, :], AO.mult)
                            if has_bv:
                                nc.vector.tensor_scalar(
                                    out=av_view, in0=av_view,
                                    scalar1=par[hb:hb + DK, 4, t:t + 1],
                                    scalar2=0.0, op0=AO.add, op1=AO.bypass)
                    attns[b] = attn

                def emit_oproj(b):
                    attn = attns[b]
                    if b == 0:
                        wo_s = wpool.tile([P, NCH, C], F32R, tag="wvo", name="wo_s")
                        nc.sync.dma_start(
                            wo_s[:], wo_d.rearrange("(k p) o -> p k o", p=P).bitcast(F32R))
                        w_t["o"] = wo_s
                    for oc in range(NCH):
                        for ih in range(2):
                            ps = ps_proj.tile([P, 512], F32, tag="proj", name="pso")
                            for ic in range(NCH):
                                nc.tensor.matmul(
                                    ps[:], w_t["o"][:, ic, oc * P:(oc + 1) * P],
                                    attn[:, ic, ih * 512:(ih + 1) * 512],
                                    start=(ic == 0), stop=(ic == NCH - 1))
                            osb = opool.tile([P, 512], F32, tag="osb", name="osb")
                            if has_bo:
                                nc.vector.tensor_scalar(
                                    out=ps[:], in0=ps[:], scalar1=par[:, 5, oc:oc + 1],
                                    scalar2=0.0, op0=AO.add, op1=AO.bypass)
                            nc.vector.tensor_tensor(
                                osb[:], ps[:], x_t[b][:, oc, ih * 512:(ih + 1) * 512],
                                AO.add)
                            nc.sync.dma_start(
                                out_d[b, oc * P:(oc + 1) * P, ih * 512:(ih + 1) * 512],
                                osb[:])

                emit_proj(0)
                emit_attn(0)
                emit_proj(1)
                emit_oproj(0)
                emit_attn(1)
                emit_oproj(1)
    nc.finalize()
    return nc


_CACHE = {}
last_run = None


def _program(flags, reps=1):
    key = (flags, reps)
    if key not in _CACHE:
        _CACHE[key] = _build(flags, reps)
    return _CACHE[key]


def prepare_inputs(x, gn_w, gn_b, conv_w, conv_b, wq, bq, wk, bk, wv, bv, wo, bo):
    x = np.ascontiguousarray(np.asarray(x, np.float32))
    f8 = lambda a: np.asarray(a, np.float64)
    # fold the 1x1 conv into the Q/K/V projections (exact algebra, float64)
    wq_f = f8(wq) @ f8(conv_w)
    wk_f = f8(wk) @ f8(conv_w)
    wv_f = f8(wv) @ f8(conv_w)
    bq_f = f8(wq) @ f8(conv_b) + f8(bq)
    bk_f = f8(wk) @ f8(conv_b) + f8(bk)
    bv_f = f8(wv) @ f8(conv_b) + f8(bv)

    par = np.zeros((6, C), np.float32)
    par[0] = np.asarray(gn_w, np.float32)
    par[1] = np.asarray(gn_b, np.float32)
    par[2] = bq_f.astype(np.float32)
    par[3] = bk_f.astype(np.float32)
    par[4] = bv_f.astype(np.float32)
    par[5] = np.asarray(bo, np.float32)

    flags = (
        bool(np.any(par[2])), bool(np.any(par[3])), bool(np.any(par[4])),
        bool(np.any(par[5])), bool(np.any(par[0] != 1.0)), bool(np.any(par[1])),
    )

    gnsel = np.zeros((P, GPC), np.float32)
    gnsel[np.arange(P), np.arange(P) // GSIZE] = 1.0
    shared = dict(
        wq=_round_fp32r(wq_f.T), wk=_round_fp32r(wk_f.T), wv=_round_fp32r(wv_f.T),
        wo=_round_fp32r(f8(wo).T), par=par, ones=np.ones((P, GPC), np.float32),
        onesrow=np.ones((1, DK), np.float32),
        gnsel=gnsel, gnbsel=np.ascontiguousarray(gnsel.T))
    xr = x.reshape(NCORES, B, C, L)
    in_maps = [dict(x=np.ascontiguousarray(xr[c]), **shared) for c in range(NCORES)]
    return flags, in_maps


def run(flags, in_maps, reps=1):
    global last_run
    nc = _program(flags, reps)
    res = run_bass_kernel_spmd(nc, in_maps, core_ids=list(range(NCORES)))
    last_run = res
    return res


def kernel(x, gn_w, gn_b, conv_w, conv_b, wq, bq, wk, bk, wv, bv, wo, bo):
    flags, in_maps = prepare_inputs(x, gn_w, gn_b, conv_w, conv_b,
                                    wq, bq, wk, bk, wv, bv, wo, bo)
    res = run(flags, in_maps, reps=1)
    out = np.concatenate([r["out"] for r in res.results], axis=0)
    return out.reshape(NCORES * B, C, 32, 32).astype(np.float32)



# revision 1
# speedup vs baseline: 1.1864x; 1.1864x over previous
"""Trainium2 Bass kernel for nn_AttentionBlock (GroupNorm + 1x1conv + MHA + residual).

Strategy:
  - Data-parallel over batch: 16 batches -> 8 NeuronCores x 2 batches. No collectives.
  - Host-side: fuse the 1x1 conv into the Q/K/V projections (float64 algebra),
    pre-transpose all weights to [in, out] and pre-round them to fp32r
    (11-bit-mantissa float the PE consumes at full speed, 1 col/cycle).
  - On-chip layout is channel-major [C, L] per batch. Attention computes
    S^T = K^T Q directly (j on partitions), exp on ScalarE straight out of
    PSUM over [128,1024] tiles (scores are bounded, so no max-subtraction is
    needed), and the softmax denominator comes for free as a ones-row
    appended to V^T in the AV matmul. Per-column normalization is done by
    bouncing the reciprocal row through DRAM with a partition-broadcast DMA.
    All matmuls are fp32r (rel err ~1e-4).
"""

import numpy as np

import concourse.bass as bass
import concourse.tile as tile
from concourse import bacc, mybir
from concourse.bass_utils import run_bass_kernel_spmd

P = 128
C = 512
L = 1024
B = 2          # batches per core
NCORES = 8
NH = 8
DK = 64
NCH = C // P   # 4 channel chunks of 128
GPC = 8        # groups per 128-chunk (16 channels per group)
GSIZE = 16
EPS = 1e-5
SCALE = float(DK) ** -0.5
F32 = mybir.dt.float32
F32R = mybir.dt.float32r
AO = mybir.AluOpType


def _round_fp32r(a: np.ndarray) -> np.ndarray:
    """Round-to-nearest-even to 11-bit mantissa (fp32r), keep fp32 layout."""
    b = np.ascontiguousarray(a, np.float32).view(np.uint32)
    r = (b.astype(np.uint64) + 0x7FF + ((b >> 12) & 1)).astype(np.uint32)
    return (r & np.uint32(0xFFFFF000)).view(np.float32)


def _build(flags, reps=1):
    has_bq, has_bk, has_bv, has_bo, has_gnw, has_gnb = flags
    nc = bacc.Bacc("TRN2", target_bir_lowering=False)

    x_d = nc.dram_tensor("x", [B, C, L], F32, kind="ExternalInput")
    wq_d = nc.dram_tensor("wq", [C, C], F32, kind="ExternalInput")   # [in, out], fp32r bits
    wk_d = nc.dram_tensor("wk", [C, C], F32, kind="ExternalInput")
    wv_d = nc.dram_tensor("wv", [C, C], F32, kind="ExternalInput")
    wo_d = nc.dram_tensor("wo", [C, C], F32, kind="ExternalInput")
    # params rows: 0 gn_w, 1 gn_b, 2 bq, 3 bk, 4 bv, 5 bo
    par_d = nc.dram_tensor("par", [6, C], F32, kind="ExternalInput")
    ones_d = nc.dram_tensor("ones", [P, GPC], F32, kind="ExternalInput")
    onesrow_d = nc.dram_tensor("onesrow", [1, DK], F32, kind="ExternalInput")
    gnsel_d = nc.dram_tensor("gnsel", [P, GPC], F32, kind="ExternalInput")
    gnbsel_d = nc.dram_tensor("gnbsel", [GPC, P], F32, kind="ExternalInput")
    out_d = nc.dram_tensor("out", [B, C, L], F32, kind="ExternalOutput")

    with tile.TileContext(nc) as tc:
        with (
            nc.allow_low_precision(reason="fp32r rounding of matmul inputs is intentional"),
            tc.tile_pool(name="weights", bufs=1) as wpool,
            tc.tile_pool(name="xpool", bufs=1) as xpool,
            tc.tile_pool(name="acts", bufs=1) as apool,
            tc.tile_pool(name="qk2", bufs=2) as qkpool,
            tc.tile_pool(name="xn", bufs=1) as xnpool,
            tc.tile_pool(name="pt", bufs=5) as ptpool,
            tc.tile_pool(name="small", bufs=1) as spool,
            tc.tile_pool(name="osb", bufs=5) as opool,
            tc.tile_pool(name="recb", bufs=4) as rpool,
            tc.tile_pool(name="ps_big", bufs=2, space="PSUM") as ps_big,
            tc.tile_pool(name="ps_av", bufs=2, space="PSUM") as ps_av,
            tc.tile_pool(name="ps_proj", bufs=2, space="PSUM") as ps_proj,
            tc.tile_pool(name="dram", bufs=4, space="DRAM") as dpool,
        ):
            # ---- x[b0] first (GN gates the pipeline), then q/k weights, then x[b1]
            x_t = []
            for b in range(B):
                xt = xpool.tile([P, NCH, L], F32, tag=f"x{b}")
                x_t.append(xt)

            def load_x(b):
                xr = x_d[b].rearrange("(k p) l -> p k l", p=P)
                for c in range(NCH):
                    nc.sync.dma_start(x_t[b][:, c, :], xr[:, c, :])

            load_x(0)
            w_t = {}
            for name, d in (("q", wq_d), ("k", wk_d)):
                w = wpool.tile([P, NCH, C], F32R, tag=f"w{name}")
                nc.sync.dma_start(w[:], d.rearrange("(k p) o -> p k o", p=P).bitcast(F32R))
                w_t[name] = w
            load_x(1)
            par = spool.tile([P, 6, NCH], F32, tag="par")
            nc.sync.dma_start(par[:], par_d.rearrange("j (k p) -> p j k", p=P))
            ones8 = spool.tile([P, GPC], F32, tag="ones8")
            nc.sync.dma_start(ones8[:], ones_d[:, :])
            onesrow = spool.tile([1, DK], F32R, tag="onesrow")
            nc.sync.dma_start(onesrow[:], onesrow_d[:, :].bitcast(F32R))
            gnsel = spool.tile([P, GPC], F32R, tag="gnsel")
            nc.sync.dma_start(gnsel[:], gnsel_d[:, :].bitcast(F32R))
            gnbsel = spool.tile([GPC, P], F32R, tag="gnbsel")
            nc.sync.dma_start(gnbsel[:], gnbsel_d[:, :].bitcast(F32R))
            eps8 = spool.tile([GPC, 1], F32, tag="eps8")
            nc.vector.memset(eps8[:], EPS)

            for rep in range(reps):
                # ---- GroupNorm statistics (per batch) ----
                rstd_pc, mean_pc, beta_pc = [], [], []
                for b in range(B):
                    # raw sums: DVE sum(x), ACT sum(x^2) via accum_out
                    rhs_f = spool.tile([P, 2 * NCH], F32, tag="gnrhs_f", name="rhs_f")
                    for c in range(NCH):
                        nc.vector.reduce_sum(rhs_f[:, c:c + 1], x_t[b][:, c, :],
                                             axis=mybir.AxisListType.X)
                        scr = ptpool.tile([P, L], F32, tag="pt", name="scr")
                        nc.scalar.activation(scr[:], x_t[b][:, c, :],
                                             mybir.ActivationFunctionType.Square,
                                             accum_out=rhs_f[:, NCH + c:NCH + c + 1])
                    rhs_r = spool.tile([P, 2 * NCH], F32R, tag="gnrhs_r", name="rhs_r")
                    nc.vector.tensor_copy(rhs_r[:], rhs_f[:])

                    gstat = ps_proj.tile([P, 512], F32, tag="proj",
                                         name="gstat")[0:GPC, 0:2 * NCH]
                    nc.tensor.matmul(gstat[:], gnsel[:], rhs_r[:], start=True, stop=True)

                    gmean = spool.tile([GPC, NCH], F32, tag="gmean", name="gmean")
                    nc.scalar.mul(gmean[:], gstat[:, 0:NCH], 1.0 / (GSIZE * L))
                    gm2 = spool.tile([GPC, NCH], F32, tag="gm2", name="gm2")
                    nc.vector.tensor_mul(gm2[:], gmean[:], gmean[:])
                    gvar = spool.tile([GPC, NCH], F32, tag="gvar", name="gvar")
                    nc.vector.scalar_tensor_tensor(
                        out=gvar[:], in0=gstat[:, NCH:2 * NCH], scalar=1.0 / (GSIZE * L),
                        in1=gm2[:], op0=AO.mult, op1=AO.subtract)
                    bvals = spool.tile([GPC, 2 * NCH], F32R, tag="bvals", name="bvals")
                    gstd = spool.tile([GPC, NCH], F32, tag="gstd", name="gstd")
                    nc.scalar.activation(gstd[:], gvar[:],
                                         mybir.ActivationFunctionType.Sqrt,
                                         bias=eps8[:], scale=1.0)
                    nc.vector.reciprocal(bvals[:, 0:NCH], gstd[:])
                    nc.vector.tensor_copy(bvals[:, NCH:2 * NCH], gmean[:])

                    bc = ps_proj.tile([P, 512], F32, tag="proj", name="bc")[:, 0:2 * NCH]
                    nc.tensor.matmul(bc[:], gnbsel[:], bvals[:], start=True, stop=True)
                    rp = spool.tile([P, NCH], F32, tag=f"rstd{b}", name="rp")
                    mp = spool.tile([P, NCH], F32, tag=f"mean{b}", name="mp")
                    if has_gnw:
                        nc.vector.tensor_tensor(rp[:], bc[:, 0:NCH], par[:, 0, :], AO.mult)
                    else:
                        nc.vector.tensor_copy(rp[:], bc[:, 0:NCH])
                    nc.scalar.copy(mp[:], bc[:, NCH:2 * NCH])
                    rstd_pc.append(rp)
                    mean_pc.append(mp)
                    if has_gnb:
                        bp = spool.tile([P, NCH], F32, tag=f"beta{b}", name="bp")
                        nc.vector.tensor_mul(bp[:], mp[:], rp[:])
                        nc.vector.tensor_tensor(bp[:], par[:, 1, :], bp[:], AO.subtract)
                        beta_pc.append(bp)
                    else:
                        beta_pc.append(None)

                # ================= pipeline =================
                prep = {}
                attns = {}

                def emit_proj(b):
                    # GN apply -> xn (fp32r)
                    xn = xnpool.tile([P, NCH, L], F32R, tag="xn", name="xn")
                    for c in range(NCH):
                        if has_gnb:
                            nc.vector.tensor_scalar(
                                out=xn[:, c, :], in0=x_t[b][:, c, :],
                                scalar1=rstd_pc[b][:, c:c + 1],
                                scalar2=beta_pc[b][:, c:c + 1],
                                op0=AO.mult, op1=AO.add)
                        else:
                            nc.vector.tensor_scalar(
                                out=xn[:, c, :], in0=x_t[b][:, c, :],
                                scalar1=mean_pc[b][:, c:c + 1],
                                scalar2=rstd_pc[b][:, c:c + 1],
                                op0=AO.subtract, op1=AO.mult)

                    # Q, K projections: [C, L] channel-major
                    qk = {}
                    for pname in ("q", "k"):
                        qk[pname] = qkpool.tile([P, NCH, L], F32R, tag=pname, name=pname)
                    for oc in range(NCH):  # interleave q/k so head 0 unblocks early
                        for pname, has_b, prow in (("q", has_bq, 2), ("k", has_bk, 3)):
                            dst = qk[pname]
                            for ih in range(2):
                                ps = ps_proj.tile([P, 512], F32, tag="proj", name="psp")
                                for ic in range(NCH):
                                    nc.tensor.matmul(
                                        ps[:], w_t[pname][:, ic, oc * P:(oc + 1) * P],
                                        xn[:, ic, ih * 512:(ih + 1) * 512],
                                        start=(ic == 0), stop=(ic == NCH - 1))
                                dv = dst[:, oc, ih * 512:(ih + 1) * 512]
                                if has_b:
                                    nc.vector.tensor_scalar(
                                        out=dv, in0=ps[:], scalar1=par[:, prow, oc:oc + 1],
                                        scalar2=0.0, op0=AO.add, op1=AO.bypass)
                                else:
                                    nc.vector.tensor_copy(dv, ps[:])

                    # V^T: [L, C] token-major with ones column per head
                    if b == 0:
                        wv_s = wpool.tile([P, NCH, C], F32R, tag="wvo", name="wv_s")
                        nc.sync.dma_start(
                            wv_s[:], wv_d.rearrange("(k p) o -> p k o", p=P).bitcast(F32R))
                        w_t["v"] = wv_s
                    vT = []
                    for lb in range(NH):
                        vt = apool.tile([P, NH, DK + 1], F32R, tag=f"vT{lb}", name="vt")
                        ps = ps_proj.tile([P, 512], F32, tag="proj", name="psv")
                        for ic in range(NCH):
                            nc.tensor.matmul(
                                ps[:], xn[:, ic, lb * P:(lb + 1) * P], w_t["v"][:, ic, :],
                                start=(ic == 0), stop=(ic == NCH - 1))
                        nc.vector.tensor_copy(vt[:, :, 0:DK],
                                               ps[:].rearrange("p (h d) -> p h d", d=DK))
                        nc.vector.tensor_copy(vt[:, :, DK], ones8[:])
                        vT.append(vt)
                    prep[b] = (qk, vT)

                def emit_attn(b):
                    qk, vT = prep[b]
                    attn = apool.tile([P, NCH, L], F32R, tag="attn", name="attn")
                    q_t, k_t = qk["q"], qk["k"]
                    for h in range(NH):
                        hb = (h % 2) * DK
                        t = h // 2
                        avs = [ps_av.tile([P, 512], F32, tag="av", name=f"av{ih}")
                               for ih in range(2)]
                        for jb in range(NH):
                            sps = ps_big.tile([P, L], F32, tag="s", name="sps")
                            for ih in range(2):
                                nc.tensor.matmul(
                                    sps[:, ih * 512:(ih + 1) * 512],
                                    k_t[hb:hb + DK, t, jb * P:(jb + 1) * P],
                                    q_t[hb:hb + DK, t, ih * 512:(ih + 1) * 512],
                                    start=True, stop=True)
                            pt = ptpool.tile([P, L], F32R, tag="pt", name="pt")
                            nc.scalar.activation(pt[:], sps[:],
                                                 mybir.ActivationFunctionType.Exp,
                                                 scale=SCALE)
                            for ih in range(2):
                                nc.tensor.matmul(
                                    avs[ih][0:DK + 1, :], vT[jb][:, h, :],
                                    pt[:, ih * 512:(ih + 1) * 512],
                                    start=(jb == 0), stop=(jb == NH - 1))
                        for ih in range(2):
                            av = avs[ih]
                            rec = rpool.tile([1, 512], F32, tag="rb", name="rec")
                            nc.vector.reciprocal(rec[:], av[DK:DK + 1, :])
                            av_view = attn[hb:hb + DK, t, ih * 512:(ih + 1) * 512]
                            nc.vector.tensor_copy(av_view, av[0:DK, :])
                            scr = dpool.tile([1, 512], F32, tag="scr", name="scr")
                            nc.sync.dma_start(scr[:], rec[:])
                            recb = rpool.tile([P, 512], F32, tag="rb", name="recb")
                            nc.sync.dma_start(recb[:], scr[:].to_broadcast((P, 512)))
                            nc.vector.tensor_tensor(av_view, av_view,
                                                    recb[hb:hb + DK, :], AO.mult)
                            if has_bv:
                                nc.vector.tensor_scalar(
                                    out=av_view, in0=av_view,
                                    scalar1=par[hb:hb + DK, 4, t:t + 1],
                                    scalar2=0.0, op0=AO.add, op1=AO.bypass)
                    attns[b] = attn

                def emit_oproj(b):
                    attn = attns[b]
                    if b == 0:
                        wo_s = wpool.tile([P, NCH, C], F32R, tag="wvo", name="wo_s")
                        nc.sync.dma_start(
                            wo_s[:], wo_d.rearrange("(k p) o -> p k o", p=P).bitcast(F32R))
                        w_t["o"] = wo_s
                    for oc in range(NCH):
                        for ih in range(2):
                            ps = ps_proj.tile([P, 512], F32, tag="proj", name="pso")
                            for ic in range(NCH):
                                nc.tensor.matmul(
                                    ps[:], w_t["o"][:, ic, oc * P:(oc + 1) * P],
                                    attn[:, ic, ih * 512:(ih + 1) * 512],
                                    start=(ic == 0), stop=(ic == NCH - 1))
                            osb = opool.tile([P, 512], F32, tag="osb", name="osb")
                            if has_bo:
                                nc.vector.tensor_scalar(
                                    out=ps[:], in0=ps[:], scalar1=par[:, 5, oc:oc + 1],
                                    scalar2=0.0, op0=AO.add, op1=AO.bypass)
                            nc.vector.tensor_tensor(
                                osb[:], ps[:], x_t[b][:, oc, ih * 512:(ih + 1) * 512],
                                AO.add)
                            nc.sync.dma_start(
                                out_d[b, oc * P:(oc + 1) * P, ih * 512:(ih + 1) * 512],
                                osb[:])

                emit_proj(0)
                emit_attn(0)
                emit_proj(1)
                emit_oproj(0)
                emit_attn(1)
                emit_oproj(1)
    nc.finalize()
    return nc


_CACHE = {}
last_run = None


def _program(flags, reps=1):
    key = (flags, reps)
    if key not in _CACHE:
        _CACHE[key] = _build(flags, reps)
    return _CACHE[key]


def prepare_inputs(x, gn_w, gn_b, conv_w, conv_b, wq, bq, wk, bk, wv, bv, wo, bo):
    x = np.ascontiguousarray(np.asarray(x, np.float32))
    f8 = lambda a: np.asarray(a, np.float64)
    # fold the 1x1 conv into the Q/K/V projections (exact algebra, float64)
    wq_f = f8(wq) @ f8(conv_w)
    wk_f = f8(wk) @ f8(conv_w)
    wv_f = f8(wv) @ f8(conv_w)
    bq_f = f8(wq) @ f8(conv_b) + f8(bq)
    bk_f = f8(wk) @ f8(conv_b) + f8(bk)
    bv_f = f8(wv) @ f8(conv_b) + f8(bv)

    par = np.zeros((6, C), np.float32)
    par[0] = np.asarray(gn_w, np.float32)
    par[1] = np.asarray(gn_b, np.float32)
    par[2] = bq_f.astype(np.float32)
    par[3] = bk_f.astype(np.float32)
    par[4] = bv_f.astype(np.float32)
    par[5] = np.asarray(bo, np.float32)

    flags = (
        bool(np.any(par[2])), bool(np.any(par[3])), bool(np.any(par[4])),
        bool(np.any(par[5])), bool(np.any(par[0] != 1.0)), bool(np.any(par[1])),
    )

    gnsel = np.zeros((P, GPC), np.float32)
    gnsel[np.arange(P), np.arange(P) // GSIZE] = 1.0
    shared = dict(
        wq=_round_fp32r(wq_f.T), wk=_round_fp32r(wk_f.T), wv=_round_fp32r(wv_f.T),
        wo=_round_fp32r(f8(wo).T), par=par, ones=np.ones((P, GPC), np.float32),
        onesrow=np.ones((1, DK), np.float32),
        gnsel=gnsel, gnbsel=np.ascontiguousarray(gnsel.T))
    xr = x.reshape(NCORES, B, C, L)
    in_maps = [dict(x=np.ascontiguousarray(xr[c]), **shared) for c in range(NCORES)]
    return flags, in_maps


def run(flags, in_maps, reps=1):
    global last_run
    nc = _program(flags, reps)
    res = run_bass_kernel_spmd(nc, in_maps, core_ids=list(range(NCORES)))
    last_run = res
    return res


def kernel(x, gn_w, gn_b, conv_w, conv_b, wq, bq, wk, bk, wv, bv, wo, bo):
    flags, in_maps = prepare_inputs(x, gn_w, gn_b, conv_w, conv_b,
                                    wq, bq, wk, bk, wv, bv, wo, bo)
    res = run(flags, in_maps, reps=1)
    out = np.concatenate([r["out"] for r in res.results], axis=0)
    return out.reshape(NCORES * B, C, 32, 32).astype(np.float32)

